# revision 1
# baseline (speedup 1.0000x reference)
"""Trainium2 Bass kernel for nn_Decoder_attri_z (sparse conv decoder chain).

Five chained gather-GEMM-scatter sparse convolutions, SPMD across 8
NeuronCores. Sharding: each layer's kernel-map pairs are partitioned by
OWNER OF THE OUTPUT ROW (out_idx // (n_out/8)); weights are replicated.
Between layers the full feature matrix is reassembled on the host (the
layer launches are independent SPMD kernels).

Per layer, per core (two phases through a DRAM contrib scratch):
  Phase A (pairs grouped by kernel offset k, padded to 128-multiples):
    indirect-DMA gather of 128 input rows -> PE transpose -> matmul with
    W_k -> contrib rows written to DRAM in pair order (batched DMA).
  Phase B (pairs sorted by output row, segmented per 128-row out tile):
    indirect-DMA gather of contrib rows -> one-hot selection matrix via
    iota/is_equal -> PE matmul-scatter accumulating in PSUM -> bias (+
    ReLU) -> output shard.
"""

import os
import numpy as np

import concourse.bass as bass
import concourse.mybir as mybir
import concourse.tile as tile
from concourse.bass_utils import run_bass_kernel_spmd
from concourse.masks import make_identity

P = 128
N_CORES = 8
TRACE = False          # set True (e.g. from test.py) to collect exec_time_ns
LAST_EXEC_NS = []      # per-launch exec_time_ns when TRACE


# ---------------------------------------------------------------- waitsplit
# This walrus build accepts at most ONE sync wait per instruction; Tile
# attaches several. Hoist extras onto same-engine NoOps placed before.
_wsctr = [0]


def _split_sync_waits(nc):
    for f in nc.m.functions:
        for bb in f.blocks:
            out = []
            for inst in bb.instructions:
                si = inst.sync_info
                waits = list(si.on_wait) if (si and si.on_wait) else []
                if len(waits) > 1:
                    for extra in waits[:-1]:
                        _wsctr[0] += 1
                        n = mybir.InstNoOp(
                            name=f"wsplit_{_wsctr[0]}", ins=[], outs=[]
                        )
                        n.engine = inst.engine
                        n.sync_info = mybir.SyncInfo(
                            on_wait=[extra], on_update=[]
                        )
                        out.append(n)
                    si.on_wait = [waits[-1]]
                out.append(inst)
            bb.instructions = out


def _ntff_shim():
    import sys, types
    if 'antenv.axon_hooks' in sys.modules:
        return
    try:
        from trn_agent_boot.trn_boot import _ntff_profile_via_ctypes
        hook = _ntff_profile_via_ctypes('/opt/axon/libaxon_pjrt.so')
    except Exception:
        hook = None
    mod = types.ModuleType('antenv.axon_hooks')
    mod.get_axon_ntff_profile_hook = lambda: hook
    mod.set_axon_ntff_profile_hook = lambda h: None
    sys.modules['antenv.axon_hooks'] = mod


# ---------------------------------------------------------------- layer cfg
LAYERS = [
    dict(name="up0", w="W_up0", b="b_up0", imap="up0_in", omap="up0_out",
         n_in=30000, n_out=100000, cin=128, cout=128, relu=False),
    dict(name="conv0", w="W_c0", b="b_c0", imap="conv0_in", omap="conv0_out",
         n_in=100000, n_out=100000, cin=128, cout=128, relu=True),
    dict(name="up1", w="W_up1", b="b_up1", imap="up1_in", omap="up1_out",
         n_in=100000, n_out=300000, cin=128, cout=128, relu=False),
    dict(name="conv1", w="W_c1", b="b_c1", imap="conv1_in", omap="conv1_out",
         n_in=300000, n_out=300000, cin=128, cout=64, relu=True),
    dict(name="conv2", w="W_c2", b="b_c2", imap="conv2_in", omap="conv2_out",
         n_in=300000, n_out=300000, cin=64, cout=3, relu=False),
]
K = 27


def _ceil(a, b):
    return -(-a // b)


# ---------------------------------------------------------------- planning
def _plan(L, in_idx, out_idx):
    """Host-side partition of kernel-map pairs. Returns per-core metadata
    with shapes uniform across cores (required for SPMD)."""
    n_out = L["n_out"]
    shard = n_out // N_CORES
    shard_pad = _ceil(shard, P) * P
    R = shard_pad // P

    fin = np.asarray(in_idx).ravel()
    fout = np.asarray(out_idx).ravel()
    fk = np.repeat(np.arange(K), in_idx.shape[1])
    core = fout // shard

    # ---- phase A: per core, pairs grouped by k; pad each k group to
    # the cross-core max tile count so the program is identical.
    percore = []
    for c in range(N_CORES):
        m = core == c
        percore.append((fin[m], fout[m] - c * shard, fk[m]))
    Tk = []
    for k in range(K):
        cnt = max(int((pk == k).sum()) for _, _, pk in percore)
        Tk.append(max(1, _ceil(cnt, P)))
    Ta = sum(Tk)
    A_k = []
    for k in range(K):
        A_k += [k] * Tk[k]

    A_rows = np.zeros((N_CORES, P, Ta), np.int32)
    slot_out = np.full((N_CORES, Ta * P), -1, np.int64)
    t0 = 0
    for k in range(K):
        for c in range(N_CORES):
            ci, co, ck = percore[c]
            sel = ck == k
            n = int(sel.sum())
            rows = np.zeros(Tk[k] * P, np.int32)
            rows[:n] = ci[sel]
            A_rows[c, :, t0:t0 + Tk[k]] = rows.reshape(Tk[k], P).T
            so = slot_out[c]
            so[t0 * P: t0 * P + n] = co[sel]
        t0 += Tk[k]

    # ---- phase B: per out tile r, slots hitting it; pad to cross-core max
    Jr = []
    sel_rc = []
    for c in range(N_CORES):
        so = slot_out[c]
        order = np.argsort(so[so >= 0] // P, kind="stable")
        valid = np.where(so >= 0)[0][order]
        rs = so[valid] // P
        # boundaries per r
        idx_by_r = np.split(valid, np.searchsorted(rs, np.arange(1, R)))
        sel_rc.append(idx_by_r)
    for r in range(R):
        cnt = max(len(sel_rc[c][r]) for c in range(N_CORES))
        Jr.append(max(1, _ceil(cnt, P)))
    Tb = sum(Jr)
    B_slot = np.zeros((N_CORES, P, Tb), np.int32)
    B_osh = np.full((N_CORES, P, Tb), -100.0, np.float32)
    j0 = 0
    for r in range(R):
        for c in range(N_CORES):
            sl = sel_rc[c][r]
            n = len(sl)
            s = np.zeros(Jr[r] * P, np.int32)
            s[:n] = sl
            o = np.full(Jr[r] * P, -100.0, np.float32)
            o[:n] = (slot_out[c][sl] - r * P).astype(np.float32)
            B_slot[c, :, j0:j0 + Jr[r]] = s.reshape(Jr[r], P).T
            B_osh[c, :, j0:j0 + Jr[r]] = o.reshape(Jr[r], P).T
        j0 += Jr[r]

    return dict(Ta=Ta, A_k=tuple(A_k), Jr=tuple(Jr), Tb=Tb, R=R,
                shard=shard, A_rows=A_rows, B_slot=B_slot, B_osh=B_osh)


# ---------------------------------------------------------------- build
_NC_CACHE = {}

WGB = 8   # contrib write batch (tiles per staging DMA)


def _build(L, Ta, A_k, Jr, Tb, R):
    key = (L["name"], Ta, A_k, Jr)
    if key in _NC_CACHE:
        return _NC_CACHE[key]
    cin, cout = L["cin"], L["cout"]
    n_in = L["n_in"]
    f32, i32 = mybir.dt.float32, mybir.dt.int32

    nc = bass.Bass()
    t_feats = nc.dram_tensor("feats", [n_in, cin], f32, kind="ExternalInput")
    t_W = nc.dram_tensor("Wt", [cin, K * cout], f32, kind="ExternalInput")
    t_bias = nc.dram_tensor("biasr", [P, cout], f32, kind="ExternalInput")
    t_Ar = nc.dram_tensor("A_rows", [P, Ta], i32, kind="ExternalInput")
    t_Bs = nc.dram_tensor("B_slot", [P, Tb], i32, kind="ExternalInput")
    t_Bo = nc.dram_tensor("B_osh", [P, Tb], f32, kind="ExternalInput")
    Ta_pad = _ceil(Ta, WGB) * WGB
    t_contrib = nc.dram_tensor("contrib", [Ta_pad * P, cout], f32)
    t_out = nc.dram_tensor("out", [R * P, cout], f32, kind="ExternalOutput")

    with tile.TileContext(nc) as tc:
        with (
            tc.tile_pool(name="const", bufs=1) as constp,
            tc.tile_pool(name="meta", bufs=1) as metap,
            tc.tile_pool(name="ga", bufs=4) as gap,
            tc.tile_pool(name="gt", bufs=4) as gtp,
            tc.tile_pool(name="stage", bufs=2) as stagep,
            tc.tile_pool(name="cb", bufs=4) as cbp,
            tc.tile_pool(name="sel", bufs=4) as selp,
            tc.tile_pool(name="outs", bufs=2) as outsp,
            tc.tile_pool(name="pst", bufs=2, space="PSUM") as pstp,
            tc.tile_pool(name="psc", bufs=2, space="PSUM") as pscp,
            tc.tile_pool(name="pso", bufs=2, space="PSUM") as psop,
        ):
            ident = constp.tile([P, P], f32)
            make_identity(nc, ident[:])
            iota_i = constp.tile([P, P], i32)
            nc.gpsimd.iota(iota_i[:], [[1, P]], channel_multiplier=0)
            iota_f = constp.tile([P, P], f32)
            nc.vector.tensor_copy(out=iota_f[:], in_=iota_i[:])
            w_s = constp.tile([cin, K * cout], f32)
            nc.sync.dma_start(out=w_s[:], in_=t_W[:])
            bias_s = constp.tile([P, cout], f32)
            nc.sync.dma_start(out=bias_s[:], in_=t_bias[:])
            ar_s = metap.tile([P, Ta], i32)
            nc.sync.dma_start(out=ar_s[:], in_=t_Ar[:])
            bs_s = metap.tile([P, Tb], i32)
            nc.sync.dma_start(out=bs_s[:], in_=t_Bs[:])
            bo_s = metap.tile([P, Tb], f32)
            nc.sync.dma_start(out=bo_s[:], in_=t_Bo[:])

            # ---------------- phase A ----------------
            for g in range(_ceil(Ta, WGB)):
                nt = min(WGB, Ta - g * WGB)
                stage = stagep.tile([P, WGB * cout], f32, tag="stage")
                for j in range(nt):
                    t = g * WGB + j
                    gbuf = gap.tile([P, cin], f32, tag="ga")
                    nc.gpsimd.indirect_dma_start(
                        out=gbuf[:], out_offset=None, in_=t_feats[:],
                        in_offset=bass.IndirectOffsetOnAxis(
                            ap=ar_s[:, t:t + 1], axis=0))
                    trp = pstp.tile([P, P], f32, space="PSUM", tag="pst")
                    nc.tensor.transpose(
                        out=trp[:cin, :], in_=gbuf[:], identity=ident[:])
                    gT = gtp.tile([cin, P], f32, tag="gt")
                    nc.vector.tensor_copy(out=gT[:], in_=trp[:cin, :])
                    ctp = pscp.tile([P, cout], f32, space="PSUM", tag="psc")
                    kk = A_k[t]
                    nc.tensor.matmul(
                        out=ctp[:], lhsT=gT[:],
                        rhs=w_s[:, kk * cout:(kk + 1) * cout],
                        start=True, stop=True)
                    nc.scalar.copy(
                        out=stage[:, j * cout:(j + 1) * cout], in_=ctp[:])
                dst = t_contrib[g * WGB * P:(g * WGB + nt) * P, :].rearrange(
                    "(j p) c -> p j c", p=P)
                src = stage[:, :nt * cout].rearrange(
                    "p (j c) -> p j c", j=nt)
                nc.sync.dma_start(out=dst, in_=src)

            # ---------------- phase B ----------------
            jcur = 0
            for r in range(R):
                J = Jr[r]
                outp = psop.tile([P, cout], f32, space="PSUM", tag="pso")
                for j in range(J):
                    cbuf = cbp.tile([P, cout], f32, tag="cb")
                    nc.gpsimd.indirect_dma_start(
                        out=cbuf[:], out_offset=None, in_=t_contrib[:],
                        in_offset=bass.IndirectOffsetOnAxis(
                            ap=bs_s[:, jcur + j:jcur + j + 1], axis=0))
                    S = selp.tile([P, P], f32, tag="sel")
                    nc.vector.tensor_tensor(
                        out=S[:],
                        in0=bo_s[:, jcur + j:jcur + j + 1].to_broadcast([P, P]),
                        in1=iota_f[:], op=mybir.AluOpType.is_equal)
                    nc.tensor.matmul(
                        out=outp[:], lhsT=S[:], rhs=cbuf[:],
                        start=(j == 0), stop=(j == J - 1))
                outs = outsp.tile([P, cout], f32, tag="outs")
                nc.vector.tensor_tensor(
                    out=outs[:], in0=outp[:], in1=bias_s[:],
                    op=mybir.AluOpType.add)
                if L["relu"]:
                    nc.scalar.activation(
                        out=outs[:], in_=outs[:],
                        func=mybir.ActivationFunctionType.Relu)
                nc.sync.dma_start(
                    out=t_out[r * P:(r + 1) * P, :], in_=outs[:])
                jcur += J

    _split_sync_waits(nc)
    _NC_CACHE[key] = nc
    return nc


# ---------------------------------------------------------------- driver
def _run_layer(L, feats, W, b, in_idx, out_idx):
    plan = _plan(L, in_idx, out_idx)
    nc = _build(L, plan["Ta"], plan["A_k"], plan["Jr"], plan["Tb"],
                plan["R"])
    cin, cout = L["cin"], L["cout"]
    Wt = np.ascontiguousarray(
        np.asarray(W).transpose(1, 0, 2).reshape(cin, K * cout),
        dtype=np.float32)
    bias_rep = np.broadcast_to(
        np.asarray(b, np.float32), (P, cout)).copy()
    feats = np.ascontiguousarray(feats, dtype=np.float32)
    in_maps = []
    for c in range(N_CORES):
        in_maps.append({
            "feats": feats, "Wt": Wt, "biasr": bias_rep,
            "A_rows": np.ascontiguousarray(plan["A_rows"][c]),
            "B_slot": np.ascontiguousarray(plan["B_slot"][c]),
            "B_osh": np.ascontiguousarray(plan["B_osh"][c]),
        })
    if TRACE:
        _ntff_shim()
    res = run_bass_kernel_spmd(nc, in_maps, core_ids=list(range(N_CORES)),
                               trace=TRACE)
    if TRACE:
        LAST_EXEC_NS.append(res.exec_time_ns)
    shard = plan["shard"]
    out = np.concatenate(
        [res.results[c]["out"][:shard] for c in range(N_CORES)], axis=0)
    return out


def kernel(x, W_up0, b_up0, W_c0, b_c0, W_up1, b_up1, W_c1, b_c1,
           W_c2, b_c2, up0_in, up0_out, conv0_in, conv0_out,
           up1_in, up1_out, conv1_in, conv1_out, conv2_in, conv2_out):
    LAST_EXEC_NS.clear()
    h = _run_layer(LAYERS[0], x, W_up0, b_up0, up0_in, up0_out)
    cls0 = _run_layer(LAYERS[1], h, W_c0, b_c0, conv0_in, conv0_out)
    h = _run_layer(LAYERS[2], cls0, W_up1, b_up1, up1_in, up1_out)
    cls1 = _run_layer(LAYERS[3], h, W_c1, b_c1, conv1_in, conv1_out)
    out = _run_layer(LAYERS[4], cls1, W_c2, b_c2, conv2_in, conv2_out)
    return (out, cls1, cls0)
